# revision 10
# baseline (speedup 1.0000x reference)
"""HSMNet feature-volume kernel for Trainium2 (Bass/Tile), 8 NeuronCores.

cost[b,c,d,h,x] = |ref[b,c,h,x] - tgt[b,c,h,x-d]| for x >= d else 0,
for 4 pyramid scales. Output-write-bound: ~216 MB f32 total output.

Sharding: core i = (batch b = i//2, channel half = i%2). Every core runs an
identical program over its [8, H, W] slices; host reassembles.

Per-scale layout: partition dim = (c_sub, h) (c_sub channels stacked to
fill partitions), free dims = (g, d, x). One VectorE subtract per
(buffer, g, d), one whole-buffer ScalarE Abs, zero-triangle memsets, and
trapezoid / (c,d)-merged rect DMAs out (3-dim AP limit respected).
"""

import numpy as np

import concourse.bass as bass
import concourse.tile as tile
from concourse import bacc, mybir
from concourse.bass_utils import run_bass_kernel_spmd

F32 = mybir.dt.float32

# (name, C_per_core, H, W, D, c_sub, g_free, d_chunk)
#   c_sub: channels stacked into partitions (P = c_sub * H)
#   g_free: channels in the free dim of one buffer
#   d_chunk: d-values per buffer (== D -> full-D buffer, (c,d)-merged rect
#            DMA possible when c_sub == 1)
FULL_SCALES = [
    ("o3", 8, 96, 160, 48, 1, 2, 24),
    ("o4", 8, 48, 80, 24, 2, 4, 24),
    ("o5", 8, 24, 40, 12, 1, 8, 12),
    ("o6", 8, 12, 20, 6, 1, 8, 6),
]
# per-scale (ref, tgt) input names
SCALE_INPUTS = {"o3": ("r1", "t1"), "o4": ("r2", "t2"),
                "o5": ("r3", "t3"), "o6": ("r4", "t4")}

N_CORES = 8
OUT_BUFS = 3
MEMSET_ENGINE = "gpsimd"


def build_nc(scales=FULL_SCALES, out_bufs=OUT_BUFS, memset_engine=None):
    nc = bacc.Bacc("TRN2", target_bir_lowering=False, debug=False,
                   num_devices=N_CORES)
    mseng = getattr(nc, memset_engine or MEMSET_ENGINE)

    ins = {}
    outs = {}
    for name, C, H, W, D, cs, gf, chunk in scales:
        rn, tn = SCALE_INPUTS[name]
        ins[rn] = nc.dram_tensor(rn, [C, H, W], F32, kind="ExternalInput").ap()
        ins[tn] = nc.dram_tensor(tn, [C, H, W], F32, kind="ExternalInput").ap()
        outs[name] = nc.dram_tensor(name, [C, D, H, W], F32,
                                    kind="ExternalOutput").ap()

    with tile.TileContext(nc) as tc:
        with tc.tile_pool(name="inp", bufs=1) as inp_pool, \
             tc.tile_pool(name="outp", bufs=out_bufs) as out_pool:
            # Load all inputs once: tile [P=(c_sub h), ng, w],
            # c = ci * ng + g (c_sub outer); one DMA per ci.
            sb_in = {}
            for name, C, H, W, D, cs, gf, chunk in scales:
                ng = C // cs
                for src in SCALE_INPUTS[name]:
                    t = inp_pool.tile([cs * H, ng, W], F32, name=f"sb_{src}",
                                      tag=f"sb_{src}")
                    for ci in range(cs):
                        nc.sync.dma_start(
                            out=t[ci * H:(ci + 1) * H, :, :],
                            in_=ins[src][ci * ng:(ci + 1) * ng]
                                .rearrange("c h w -> h c w"))
                    sb_in[src] = t

            for name, C, H, W, D, cs, gf, chunk in scales:
                ng = C // cs
                rt, tt = (sb_in[s] for s in SCALE_INPUTS[name])
                odram = outs[name].rearrange("c d h w -> h c d w")
                for g0 in range(0, ng, gf):
                    for d0 in range(0, D, chunk):
                        d1 = min(d0 + chunk, D)
                        nd = d1 - d0
                        buf = out_pool.tile([cs * H, gf, nd, W], F32,
                                            name=f"buf_{name}", tag="obuf")
                        if nd > 1:
                            # zero triangle inside the trapezoid:
                            # rows j>0, cols [d0, d0+j)
                            mseng.memset(buf[:, :, 1:nd, d0:d1 - 1], 0.0)
                        for d in range(d0, d1):
                            j = d - d0
                            nc.vector.tensor_sub(
                                buf[:, :, j, d:W],
                                rt[:, g0:g0 + gf, d:W],
                                tt[:, g0:g0 + gf, 0:W - d])
                        # abs in place over the DMA'd region [d0:W),
                        # split per (gi, d-half) so ACT work trails the
                        # subtracts and releases each DMA early
                        jh = [(0, nd // 2), (nd // 2, nd)] if nd >= 8 \
                            else [(0, nd)]
                        for gi in range(gf):
                            for j0, j1 in jh:
                                ap = buf[:, gi, j0:j1, d0:W]
                                nc.scalar.activation(
                                    ap, ap, mybir.ActivationFunctionType.Abs)
                        if cs == 1 and nd == D:
                            # (c,d)-merged single rect DMA for this buffer
                            nc.sync.dma_start(
                                out=odram[:, g0:g0 + gf, :, :],
                                in_=buf[:, :, :, :])
                        else:
                            for ci in range(cs):
                                for gi in range(gf):
                                    c = ci * ng + g0 + gi
                                    nc.sync.dma_start(
                                        out=odram[:, c, d0:d1, d0:W],
                                        in_=buf[ci * H:(ci + 1) * H, gi,
                                                :, d0:W])
    nc.compile()
    return nc


_NC_CACHE = {}


def _get_nc():
    if "nc" not in _NC_CACHE:
        _NC_CACHE["nc"] = build_nc()
    return _NC_CACHE["nc"]


def _in_maps(conv40, conv41, conv30, conv31, conv20, conv21, conv10, conv11):
    maps = []
    for core in range(N_CORES):
        b, half = divmod(core, 2)
        c0, c1 = half * 8, half * 8 + 8
        maps.append({
            "r1": np.ascontiguousarray(conv10[b, c0:c1]),
            "t1": np.ascontiguousarray(conv11[b, c0:c1]),
            "r2": np.ascontiguousarray(conv20[b, c0:c1]),
            "t2": np.ascontiguousarray(conv21[b, c0:c1]),
            "r3": np.ascontiguousarray(conv30[b, c0:c1]),
            "t3": np.ascontiguousarray(conv31[b, c0:c1]),
            "r4": np.ascontiguousarray(conv40[b, c0:c1]),
            "t4": np.ascontiguousarray(conv41[b, c0:c1]),
        })
    return maps


def _assemble(results):
    full = {}
    shapes = {"o3": (4, 16, 48, 96, 160), "o4": (4, 16, 24, 48, 80),
              "o5": (4, 16, 12, 24, 40), "o6": (4, 16, 6, 12, 20)}
    for name, shp in shapes.items():
        out = np.empty(shp, dtype=np.float32)
        for core in range(N_CORES):
            b, half = divmod(core, 2)
            out[b, half * 8:half * 8 + 8] = results[core][name]
        full[name] = out
    return full


def kernel(conv40, conv41, conv30, conv31, conv20, conv21, conv10, conv11,
           maxdisp=384, **run_kwargs):
    assert int(maxdisp) == 384
    nc = _get_nc()
    maps = _in_maps(conv40, conv41, conv30, conv31,
                    conv20, conv21, conv10, conv11)
    res = run_bass_kernel_spmd(nc, maps, core_ids=list(range(N_CORES)),
                               **run_kwargs)
    full = _assemble(res.results)
    kernel.last_results = res
    return (full["o6"], full["o5"], full["o4"], full["o3"])
